# revision 32
# baseline (speedup 1.0000x reference)
"""Multi-head self-attention (RoPE, causal) on 8 trn2 NeuronCores.

Sharding: batch (4) x head-group (2x8 heads) = 8 shards, one per core.
Each core: QKV projection for its 8 heads -> RoPE -> causal attention
(scores kept transposed [k, q]; softmax denominator accumulated for free
by a ones-column appended to V's stationary tile) -> partial o_proj over
its 512 head-dims, interleaved into the last head-pair's attention.
Host sums the two partial o_proj outputs of each batch pair (the
tensor-parallel all-reduce) and concatenates batches.

Engine budget notes:
- PE HAM throttle is activity-based; every PE-idle gap re-throttles to
  1.2GHz for ~3.4us, so the whole loop is built to keep the PE queue fed:
  denominators ride the PV matmuls, the causal mask is accumulated by an
  extra (maskT @ I) matmul instead of a DVE add, and the softmax
  normalize chain is short (ACT copies + one fast-reciprocal + 2 muls).
- DVE ops cost ~680ns per 512 free elems regardless of partitions or
  dtype, so RoPE runs on 1024-wide tiles and everything movable is off
  the vector engine (mask -> PE, den copies -> scalar engine).
- All input DMA is host-pre-tiled so each transfer is one descriptor set
  with 2-8KB contiguous per-partition segments; x arrives token-major so
  the V projection starts as soon as the first quarter lands.
"""
import os
import sys
import math

sys.path.insert(0, "/opt/trn_rl_repo")

import numpy as np
import ml_dtypes
from contextlib import ExitStack

import concourse.bacc as bacc
import concourse.tile as tile
from concourse import mybir
from concourse.bass_utils import run_bass_kernel_spmd
from concourse.dve_ops import (
    RECIP_APPROX_FAST_CONSTS as _RC,
    RECIPROCAL_APPROX_FAST as _RF,
)

B, S, D, H, DK = 4, 2048, 1024, 16, 64
NCORES = 8
ND = D // 128          # 8 d-tiles of the model dim
NT = S // 512          # 4 token super-blocks
NKT = S // 128         # 16 key/token 128-blocks
HPC = H // 2           # heads per core = 8
NHP = HPC // 2         # head-pairs per core = 4
F32 = mybir.dt.float32
F32R = mybir.dt.float32r
BF16 = mybir.dt.bfloat16
NEG = -30000.0
BFDT = ml_dtypes.bfloat16

_CACHE = {}


def _build():
    DBG = bool(os.environ.get("KDBG"))
    nc = bacc.Bacc("TRN2", target_bir_lowering=False, num_devices=NCORES)

    # host-pre-tiled inputs (partition dim first, contiguous per chunk)
    xT_d = nc.dram_tensor("xT", [128, NT, ND, 512], BF16, kind="ExternalInput")
    wq_d = nc.dram_tensor("wq", [128, NHP, ND, 128], BF16, kind="ExternalInput")
    wk_d = nc.dram_tensor("wk", [128, NHP, ND, 128], BF16, kind="ExternalInput")
    wv_d = nc.dram_tensor("wv", [128, ND, HPC * DK], BF16, kind="ExternalInput")
    wo_d = nc.dram_tensor("wo", [128, NHP, D], F32, kind="ExternalInput")
    ropeC_d = nc.dram_tensor("ropeC", [128, S], BF16, kind="ExternalInput")
    ropeS_d = nc.dram_tensor("ropeS", [128, S], BF16, kind="ExternalInput")
    maskT_d = nc.dram_tensor("maskT", [128, 128], BF16, kind="ExternalInput")
    ident_d = nc.dram_tensor("ident", [128, 128], BF16, kind="ExternalInput")
    yT_d = nc.dram_tensor("yT", [ND, 128, S], BF16, kind="ExternalOutput")
    if DBG:
        dV_d = nc.dram_tensor("dV", [128, NKT, HPC, 65], BF16, kind="ExternalOutput")
        dQT_d = nc.dram_tensor("dQT", [128, S], BF16, kind="ExternalOutput")
        dKT_d = nc.dram_tensor("dKT", [128, S], BF16, kind="ExternalOutput")
        dPoA_d = nc.dram_tensor("dPoA", [128, 512], F32, kind="ExternalOutput")
        dPoB_d = nc.dram_tensor("dPoB", [128, 512], F32, kind="ExternalOutput")
        dAT_d = nc.dram_tensor("dAT", [128, NHP, S], F32, kind="ExternalOutput")

    with ExitStack() as ctx:
        tc = ctx.enter_context(tile.TileContext(nc))

        const = ctx.enter_context(tc.tile_pool(name="const", bufs=1))
        xpool = ctx.enter_context(tc.tile_pool(name="x", bufs=1))
        vpool = ctx.enter_context(tc.tile_pool(name="v", bufs=1))
        qkpool = ctx.enter_context(tc.tile_pool(name="qk", bufs=2))
        wpool = ctx.enter_context(tc.tile_pool(name="w", bufs=2))
        wopool = ctx.enter_context(tc.tile_pool(name="wo", bufs=1))
        tmp = ctx.enter_context(tc.tile_pool(name="tmp", bufs=2))
        es = ctx.enter_context(tc.tile_pool(name="es", bufs=6))
        apool = ctx.enter_context(tc.tile_pool(name="a", bufs=1))
        ypool = ctx.enter_context(tc.tile_pool(name="y", bufs=2))
        ps = ctx.enter_context(tc.tile_pool(name="ps", bufs=2, space="PSUM"))
        psqp = ctx.enter_context(tc.tile_pool(name="psq", bufs=1, space="PSUM"))
        pov = ctx.enter_context(tc.tile_pool(name="pov", bufs=3, space="PSUM"))

        # ---- x streams on sync; weights on the gpsimd queue ------------
        xT = xpool.tile([128, NT, ND, 512], BF16)
        for tb in range(NT):
            nc.sync.dma_start(out=xT[:, tb], in_=xT_d[:, tb])

        ropeC = const.tile([128, S], BF16)
        ropeS = const.tile([128, S], BF16)
        wv_sb = wpool.tile([128, ND, HPC * DK], BF16, tag="wv")
        maskT = const.tile([128, 128], BF16)
        ident = const.tile([128, 128], BF16)
        ones_f = const.tile([33, 64], F32)
        nc.vector.memset(ones_f, 1.0)
        ones_r = const.tile([33, 64], BF16)
        nc.vector.tensor_copy(ones_r, ones_f)

        # V with a ones column appended per head: PV matmuls emit the
        # softmax denominator as output partition 64 for free.
        V = vpool.tile([128, NKT, HPC, 65], BF16)
        nc.vector.memset(V[:, :, :, :], 1.0)

        # attention output (normalized), o_proj consumes from SBUF
        aT = apool.tile([128, NHP, S], F32R)

        # ---- per head-pair: Q^T/K^T projection + rope + attention ------
        def proj_units(hp, QT, KT):
            """Emit-closures for one head-pair's Q/K projection, split into
            per-512-token micro-units (one 8-matmul unit, one rope unit) so
            they can be sprinkled between attention blocks as independent
            PE work that bridges the exp-wait of each block."""
            units = []
            state = {}

            def dma_unit(w_d, wtag):
                def emit():
                    wt = wpool.tile([128, ND, 128], BF16, tag=wtag)
                    nc.gpsimd.dma_start(out=wt, in_=w_d[:, hp])
                    state[wtag] = wt
                return emit

            def mm_unit(wtag, tb):
                def emit():
                    wt = state[wtag]
                    psq = psqp.tile([128, 512], F32, tag="psq")
                    for d in range(ND):
                        nc.tensor.matmul(
                            psq[:, :],
                            wt[:, d, :],
                            xT[:, tb, d, :],
                            start=(d == 0),
                            stop=(d == ND - 1),
                        )
                    state["psq"] = psq
                return emit

            def rope_unit(OUT, tb):
                def emit():
                    psq = state["psq"]
                    cs = slice(512 * tb, 512 * (tb + 1))
                    t2 = tmp.tile([128, 512], F32, tag="t2")
                    for h2 in range(2):
                        b0 = 64 * h2
                        nc.vector.tensor_mul(
                            t2[b0 : b0 + 32, :],
                            psq[b0 + 32 : b0 + 64, :],
                            ropeS[b0 : b0 + 32, cs],
                        )
                        nc.vector.tensor_mul(
                            t2[b0 + 32 : b0 + 64, :],
                            psq[b0 : b0 + 32, :],
                            ropeS[b0 + 32 : b0 + 64, cs],
                        )
                    t1 = tmp.tile([128, 512], F32, tag="t1")
                    nc.vector.tensor_mul(t1[:, :], psq[:, :], ropeC[:, cs])
                    nc.vector.tensor_add(OUT[:, cs], t1[:, :], t2[:, :])
                return emit

            for w_d, OUT, wtag in ((wq_d, QT, "wq"), (wk_d, KT, "wk")):
                units.append(dma_unit(w_d, wtag))
                for tb in range(NT):
                    units.append(mm_unit(wtag, tb))
                    units.append(rope_unit(OUT, tb))
            return units

        def make_qk(hp):
            qt_tile = qkpool.tile([128, S], BF16, tag="qt")
            kt_tile = qkpool.tile([128, S], BF16, tag="kt")
            return qt_tile, kt_tile

        # hp0's q/k weights go first on the gpsimd queue (smallest, gates
        # the first matmul), then rope tables, then wv for the V-proj.
        qk_cur = make_qk(0)
        units0 = proj_units(0, *qk_cur)
        units0[0]()  # wq dma
        units0[9]()  # wk dma
        nc.gpsimd.dma_start(out=wv_sb, in_=wv_d[:, :, :])
        nc.gpsimd.dma_start(out=ropeC, in_=ropeC_d[:, :])
        nc.gpsimd.dma_start(out=ropeS, in_=ropeS_d[:, :])
        nc.gpsimd.dma_start(out=maskT[:, :], in_=maskT_d[:, :])
        nc.gpsimd.dma_start(out=ident[:, :], in_=ident_d[:, :])
        # chunk-major: per 512-token chunk, hp0's q/k proj then V-proj,
        # so the PE always has work for whichever x chunks have landed
        for tb in range(NT):
            units0[1 + 2 * tb]()       # wq mm(tb)
            units0[2 + 2 * tb]()       # wq rope(tb)  (DVE, overlaps V-proj)
            for t in range(4 * tb, 4 * tb + 2):
                psv = pov.tile([128, 512], F32, tag="po")
                for d in range(ND):
                    nc.tensor.matmul(
                        psv[:, :],
                        xT[:, tb, d, 128 * (t % 4) : 128 * (t % 4 + 1)],
                        wv_sb[:, d, :],
                        start=(d == 0),
                        stop=(d == ND - 1),
                    )
                nc.vector.tensor_copy(V[:, t, :, 0:64], psv[:, :])
            units0[10 + 2 * tb]()      # wk mm(tb)
            units0[11 + 2 * tb]()      # wk rope(tb)
            for t in range(4 * tb + 2, 4 * tb + 4):
                psv = pov.tile([128, 512], F32, tag="po")
                for d in range(ND):
                    nc.tensor.matmul(
                        psv[:, :],
                        xT[:, tb, d, 128 * (t % 4) : 128 * (t % 4 + 1)],
                        wv_sb[:, d, :],
                        start=(d == 0),
                        stop=(d == ND - 1),
                    )
                nc.vector.tensor_copy(V[:, t, :, 0:64], psv[:, :])
        if DBG:
            nc.sync.dma_start(out=dQT_d[:, :], in_=qk_cur[0][:, :])
            nc.sync.dma_start(out=dKT_d[:, :], in_=qk_cur[1][:, :])
            nc.sync.dma_start(out=dV_d[:, :, :, :], in_=V[:, :, :, :])

        # o_proj weights: needed from hp 3 on; after hp0's q/k on gpsimd q
        wo_sb = wopool.tile([128, NHP, D], F32R)
        nc.gpsimd.dma_start(out=wo_sb, in_=wo_d[:, :, :].bitcast(F32R))

        def o_proj_unit(qb, et):
            """One 128-col tile of this core's partial o_proj for block qb."""
            def emit():
                psy_t = pov.tile([128, 512], F32, tag="po")
                qs = slice(512 * qb, 512 * (qb + 1))
                for dd in range(NHP):
                    nc.tensor.matmul(
                        psy_t[:, :],
                        wo_sb[:, dd, 128 * et : 128 * (et + 1)],
                        aT[:, dd, qs],
                        start=(dd == 0),
                        stop=(dd == NHP - 1),
                    )
                y_t = ypool.tile([128, 512], BF16, tag="y")
                nc.vector.tensor_copy(y_t[:, :], psy_t[:, :])
                nc.sync.dma_start(out=yT_d[et, :, qs], in_=y_t[:, :])
            return emit

        for hp in range(NHP):
            QT, KT = qk_cur
            last = hp == NHP - 1
            if not last:
                qk_next = make_qk(hp + 1)
                pending = list(proj_units(hp + 1, *qk_next))
                pending.reverse()  # pop() from the front
            else:
                pending = []
            slot = 0

            for qb in range(NT):
                poA = pov.tile([128, 512], F32, tag="po")
                poB = pov.tile([128, 512], F32, tag="po")
                nkb = 4 * qb + 4
                q0s = [max(0, 128 * (kb - 4 * qb)) for kb in range(nkb)]

                def emit_scores(kb):
                    q0 = q0s[kb]
                    diag = kb >= 4 * qb
                    pss = ps.tile([128, 2, 512], F32, tag="ps")
                    for h2 in range(2):
                        b0 = 64 * h2
                        nc.tensor.matmul(
                            pss[:, h2, q0:512],
                            KT[b0 : b0 + 64, 128 * kb : 128 * (kb + 1)],
                            QT[b0 : b0 + 64, 512 * qb + q0 : 512 * (qb + 1)],
                            start=True,
                            stop=not diag,
                            tile_position=(b0, 0),
                            skip_group_check=True,
                        )
                        if diag:
                            # accumulate the causal mask on the PE:
                            # pss[:, h2, q0:q0+128] += maskT.T @ I
                            nc.tensor.matmul(
                                pss[:, h2, q0 : q0 + 128],
                                maskT[:, :],
                                ident[:, :],
                                start=False,
                                stop=True,
                                tile_position=(0, 0),
                                skip_group_check=True,
                            )
                    return pss

                pss_cur = emit_scores(0)
                for kb in range(nkb):
                    q0 = q0s[kb]
                    pss = pss_cur
                    if kb + 1 < nkb:
                        pss_cur = emit_scores(kb + 1)
                    # interleave deferred work (next pair's projection, or
                    # o_proj tiles of the previous qb on the last pair)
                    slot += 1
                    if pending and (last or slot % 2 == 0) and kb < nkb - 2:
                        pending.pop()()
                    es_t = es.tile([128, 2, 512], BF16, tag="es")
                    nc.scalar.activation(
                        es_t[:, :, q0:512],
                        pss[:, :, q0:512],
                        mybir.ActivationFunctionType.Exp,
                    )
                    first = kb == 0
                    lastkb = kb == nkb - 1
                    for h2, po in ((0, poA), (1, poB)):
                        nc.tensor.matmul(
                            po[0:65, q0:512],
                            V[:, kb, 2 * hp + h2, :],
                            es_t[:, h2, q0:512],
                            start=first,
                            stop=lastkb,
                            skip_group_check=True,
                        )

                # normalize: aT = po[0:64] / po[64].  Denominator rows are
                # copied out by the scalar engine, broadcast across 64
                # partitions by tiny PE matmuls, inverted by one fast DVE
                # reciprocal directly on the PSUM broadcast.
                if DBG and hp == 0 and qb == 0:
                    dbgA = tmp.tile([128, 512], F32, tag="dbgA")
                    dbgB = tmp.tile([128, 512], F32, tag="dbgB")
                    nc.vector.tensor_copy(dbgA[:, :], poA[:, :])
                    nc.vector.tensor_copy(dbgB[:, :], poB[:, :])
                    nc.sync.dma_start(out=dPoA_d[:, :], in_=dbgA[:, :])
                    nc.sync.dma_start(out=dPoB_d[:, :], in_=dbgB[:, :])
                den_r = tmp.tile([33, 512], BF16, tag="den")
                nc.scalar.copy(den_r[0:1, :], poA[64:65, :])
                nc.scalar.copy(den_r[32:33, :], poB[64:65, :])
                psb = pov.tile([128, 512], F32, tag="po")
                nc.tensor.matmul(
                    psb[0:64, :],
                    ones_r[0:1, :],
                    den_r[0:1, :],
                    start=True,
                    stop=True,
                    tile_position=(0, 0),
                    skip_group_check=True,
                )
                nc.tensor.matmul(
                    psb[64:128, :],
                    ones_r[32:33, :],
                    den_r[32:33, :],
                    start=True,
                    stop=True,
                    tile_position=(32, 64),
                    skip_group_check=True,
                )
                recbc = tmp.tile([128, 512], F32, tag="recbc")
                nc.vector._custom_dve(
                    _RF,
                    out=recbc[:, :],
                    in0=psb[:, :],
                    s0=_RC["s0"],
                    s1=_RC["s1"],
                    imm2=_RC["imm2"],
                )
                qs = slice(512 * qb, 512 * (qb + 1))
                nc.vector.tensor_mul(
                    aT[0:64, hp, qs], poA[0:64, :], recbc[0:64, :]
                )
                nc.vector.tensor_mul(
                    aT[64:128, hp, qs], poB[0:64, :], recbc[64:128, :]
                )
                if last:
                    pending.extend(o_proj_unit(qb, et) for et in range(ND))

            if not last:
                for emit in reversed(pending):  # flush leftovers in order
                    emit()
                pending = []
                qk_cur = qk_next

        for emit in reversed(pending):  # o_proj of the final qb
            emit()
        if DBG:
            nc.sync.dma_start(out=dAT_d[:, :, :], in_=aT[:, :, :].bitcast(F32))

    nc.compile()
    return nc


_PERM = np.concatenate([np.arange(0, DK, 2), np.arange(1, DK, 2)])


def _tile_pd(w, nd):
    """[128*nd, cols] -> [128, nd, cols] (partition-major for 1-shot DMA)."""
    cols = w.shape[1]
    return np.ascontiguousarray(
        w.reshape(nd, 128, cols).transpose(1, 0, 2)
    )


def _prep_core_inputs(x, token_positions, w_qkv, w_o, core):
    b = core // 2
    h0 = HPC * (core % 2)

    xT = x[b].T.astype(BFDT)  # [D, S]
    # [128, NT, ND, 512]: chunk tb contiguous per partition
    xT_t = np.ascontiguousarray(
        xT.reshape(ND, 128, NT, 512).transpose(1, 2, 0, 3)
    )

    w_q = w_qkv[0 * D : 1 * D]
    w_k = w_qkv[1 * D : 2 * D]
    w_v = w_qkv[2 * D : 3 * D]

    def gather(w, permute, scale):
        rows = []
        for j in range(HPC):
            g = h0 + j
            blk = w[DK * g : DK * (g + 1)]
            if permute:
                blk = blk[_PERM]
            rows.append(blk)
        out = np.concatenate(rows, axis=0).astype(np.float32) * scale
        return np.ascontiguousarray(out.T)  # [D, HPC*DK]

    wq = gather(w_q, True, 1.0 / math.sqrt(DK)).astype(BFDT)
    wk = gather(w_k, True, 1.0).astype(BFDT)
    wv = gather(w_v, False, 1.0).astype(BFDT)

    # [128, NHP, ND, 128]: per-hp chunk contiguous per partition
    def qk_tile(w):
        t = _tile_pd(w, ND).reshape(128, ND, NHP, 128)
        return np.ascontiguousarray(t.transpose(0, 2, 1, 3))

    wq_t = qk_tile(wq)
    wk_t = qk_tile(wk)
    wv_t = _tile_pd(wv, ND)

    # w_o: [e_out, d_in]; take the d rows of this core's heads -> [512, D]
    rows = []
    for j in range(HPC):
        g = h0 + j
        rows.append(w_o[:, DK * g : DK * (g + 1)].T)
    wo = np.concatenate(rows, axis=0).astype(np.float32)
    wo_t = _tile_pd(wo, NHP)

    pos = token_positions.astype(np.float32)
    inv = (10000.0 ** (-(np.arange(0, DK, 2, dtype=np.float32)) / DK)).astype(
        np.float32
    )
    ang = pos[:, None] * inv[None, :]  # [S, 32]
    c = np.cos(ang).T.astype(np.float32)  # [32, S]
    s = np.sin(ang).T.astype(np.float32)
    C64 = np.concatenate([c, c], axis=0)
    S64 = np.concatenate([-s, s], axis=0)
    ropeC = np.ascontiguousarray(np.concatenate([C64, C64], axis=0)).astype(BFDT)
    ropeS = np.ascontiguousarray(np.concatenate([S64, S64], axis=0)).astype(BFDT)

    ki = np.arange(128)[:, None]
    qi = np.arange(128)[None, :]
    mask = np.where(ki <= qi, 0.0, NEG).astype(np.float32)
    maskT = np.ascontiguousarray(mask.T).astype(BFDT)
    ident = np.eye(128, dtype=np.float32).astype(BFDT)

    return {
        "xT": xT_t,
        "wq": wq_t,
        "wk": wk_t,
        "wv": wv_t,
        "wo": wo_t,
        "ropeC": ropeC,
        "ropeS": ropeS,
        "maskT": maskT,
        "ident": ident,
    }


def kernel(x, token_positions, w_qkv, w_o):
    x = np.asarray(x, dtype=np.float32)
    token_positions = np.asarray(token_positions)
    w_qkv = np.asarray(w_qkv, dtype=np.float32)
    w_o = np.asarray(w_o, dtype=np.float32)

    if "nc" not in _CACHE:
        _CACHE["nc"] = _build()
    nc = _CACHE["nc"]

    in_maps = [
        _prep_core_inputs(x, token_positions, w_qkv, w_o, c)
        for c in range(NCORES)
    ]
    res = run_bass_kernel_spmd(nc, in_maps, core_ids=list(range(NCORES)))
    _CACHE["last_results"] = res

    out = np.empty((B, S, D), dtype=np.float32)
    for b in range(B):
        yT = res.results[2 * b]["yT"].astype(np.float32) + res.results[
            2 * b + 1
        ]["yT"].astype(np.float32)
        out[b] = yT.reshape(D, S).T
    return out
